# revision 15
# baseline (speedup 1.0000x reference)
"""Multi-head attention (B=8, N=1024, E=768, H=12) on 8 trn2 NeuronCores.

Sharding: pure data-parallel over batch — core c computes batch element c
entirely (QKV projections, per-head attention, output projection), so no
collectives are needed. Returns (out, attn) exactly like the reference.

Per-core layout strategy (bf16 matmul inputs, fp32 PSUM accumulation,
fp32 softmax normalization):
  - PE contracts over the partition axis, so x (tokens-major) and W
    ([out,in]) are transposed on-chip (PE transpose-mode), giving
    x.T / W.T in bf16.
  - q.T, k.T are head-major ([o, t]); one 128-partition o-tile holds a
    PAIR of heads, so the K=64 score matmuls for the two heads run
    concurrently in disjoint PE row-groups.
  - Scores are computed TRANSPOSED (S.T = k @ q.T per head): softmax's
    reduction axis (tk) lands on partitions, where a matmul against an
    appended ones-column of V computes the softmax denominators for free:
    lhsT = [v_h | 1] gives U.T rows 0..63 = E'v and row 64 = rowsums.
  - exp runs on ScalarE straight out of PSUM (scale=1/sqrt(dh) folded in),
    emitting bf16 E.T tiles.
  - Normalizers come from an ACT ln/exp chain (no slow 1-lane DVE
    reciprocal): ln(rowsum) row -> tiny PE transposes -> exp(-ln) gives the
    per-tq reciprocal in both row and column form.
  - Attention probabilities: E.T tiles are transposed back through the PE
    (transpose-mode, bf16 PSUM) and evicted PSUM->SBUF on VectorE at 2x
    mode with the per-partition reciprocal scale.
  - Each block's epilogue (transposes/evictions/DMA) is software-pipelined
    into the NEXT block's score/exp/AV loop so the PE stream stays dense.
"""

import numpy as np
from contextlib import ExitStack

import concourse.bass as bass
import concourse.mybir as mybir
import concourse.tile as tile
from concourse import bacc
from concourse.masks import make_identity

P = 128
T = 1024          # tokens
E = 768           # embed dim
H = 12            # heads
DH = 64           # head dim
SCALE = DH ** -0.5
ET = E // P       # 6 e-tiles
TT = T // P       # 8 t-tiles
NPAIR = H // 2    # 6 head pairs
NC = 8            # cores
VW = 65           # v columns per head in vaug (64 + ones column)

FP = mybir.dt.float32
CDT = mybir.dt.bfloat16     # matmul-input dtype (PSUM accumulation is fp32)
AF = mybir.ActivationFunctionType


def _emit(ctx: ExitStack, tc: tile.TileContext, io: dict):
    nc = tc.nc

    const = ctx.enter_context(tc.tile_pool(name="const", bufs=1))
    nat_pool = ctx.enter_context(tc.tile_pool(name="nat", bufs=4))
    xT_pool = ctx.enter_context(tc.tile_pool(name="xT", bufs=2))
    wT_pool = ctx.enter_context(tc.tile_pool(name="wT", bufs=2))
    qT_pool = ctx.enter_context(tc.tile_pool(name="qT", bufs=1))
    kT_pool = ctx.enter_context(tc.tile_pool(name="kT", bufs=1))
    cT_pool = ctx.enter_context(tc.tile_pool(name="cT", bufs=1))
    vaug_pool = ctx.enter_context(tc.tile_pool(name="vaug", bufs=1))
    et_pool = ctx.enter_context(tc.tile_pool(name="et", bufs=18))
    ast_pool = ctx.enter_context(tc.tile_pool(name="ast", bufs=4))
    y_pool = ctx.enter_context(tc.tile_pool(name="y", bufs=2))
    rl_pool = ctx.enter_context(tc.tile_pool(name="rl", bufs=4))
    rr_pool = ctx.enter_context(tc.tile_pool(name="rr", bufs=4))
    ps_pool = ctx.enter_context(tc.tile_pool(name="ps", bufs=2, space="PSUM"))
    pt_pool = ctx.enter_context(tc.tile_pool(name="pt", bufs=2, space="PSUM"))
    pu_pool = ctx.enter_context(tc.tile_pool(name="pu", bufs=2, space="PSUM"))

    # ---- constants: identity, biases ------------------------------------
    ident = const.tile([P, P], CDT)
    make_identity(nc, ident[:])
    one_f32 = const.tile([1, 1], FP)
    nc.any.memset(one_f32[:], 1.0)

    bqt = const.tile([P, ET], FP)   # bq as per-partition columns per o-tile
    nc.sync.dma_start(bqt[:], io["bq"].rearrange("(o p) -> p o", p=P))
    bkt = const.tile([P, ET], FP)
    nc.sync.dma_start(bkt[:], io["bk"].rearrange("(o p) -> p o", p=P))

    bv_row = const.tile([1, E], FP)
    nc.sync.dma_start(bv_row[:], io["bv"].rearrange("(a e) -> a e", a=1))
    bv_b = const.tile([P, E], FP)
    nc.gpsimd.partition_broadcast(bv_b[:], bv_row[:])
    bo_row = const.tile([1, E], FP)
    nc.sync.dma_start(bo_row[:], io["bo"].rearrange("(a e) -> a e", a=1))
    bo_b = const.tile([P, E], FP)
    nc.gpsimd.partition_broadcast(bo_b[:], bo_row[:])

    # ---- helper: build transposed copy in SBUF --------------------------
    def build_T(dst, src_dram, R):
        """dst sbuf [P, CT, R] <- transpose of src_dram [R, C=CT*128].

        dst[p, c, r] = src[r, c*128+p]."""
        CT = dst.shape[1]
        for r in range(R // P):
            nat = nat_pool.tile([P, CT * P], CDT, tag="nat")
            # SWDGE DMA casts fp32 -> bf16 in flight
            nc.gpsimd.dma_start(nat[:], src_dram[r * P:(r + 1) * P, :])
            ps = pu_pool.tile([P, CT * P], CDT, tag="u", name=f"bt{r}")
            for c in range(CT):
                nc.tensor.transpose(
                    ps[:, c * P:(c + 1) * P],
                    nat[:, c * P:(c + 1) * P],
                    ident[:],
                )
            nc.vector.tensor_copy(
                dst[:, :, r * P:(r + 1) * P],
                ps[:].rearrange("p (c x) -> p c x", x=P),
            )

    # ---- projections ----------------------------------------------------
    # q.T / k.T: [P, ET, T] head-pair-major: dst[p, jo, t] = proj.T[jo*128+p, t]
    qT = qT_pool.tile([P, ET, T], CDT)
    kT = kT_pool.tile([P, ET, T], CDT)
    # vaug: [P, TT, H*65]: per t-tile, per head: 64 v columns + ones column
    vaug = vaug_pool.tile([P, TT, H * VW], CDT)
    nc.any.memset(vaug[:], 1.0)

    def proj_headmajor(dst, wT, xT, bias_cols):
        # dst[:, jo, t] = (x @ W.T).T rows [jo*128, jo*128+128) + bias
        for jo in range(ET):
            for t2 in range(2):
                if (jo * 2 + t2) % 2 == 0:
                    ps = ps_pool.tile([P, 1024], FP, tag="s", name=f"pp{jo}_{t2}")
                else:
                    ps = pt_pool.tile([P, 512], FP, tag="t", name=f"pp{jo}_{t2}")
                psv = ps[:, :512]
                for je in range(ET):
                    nc.tensor.matmul(
                        psv,
                        wT[:, je, jo * P:(jo + 1) * P],
                        xT[:, je, t2 * 512:(t2 + 1) * 512],
                        start=(je == 0), stop=(je == ET - 1),
                    )
                nc.vector.tensor_scalar_add(
                    dst[:, jo, t2 * 512:(t2 + 1) * 512], psv,
                    bias_cols[:, jo:jo + 1],
                )

    def proj_v(wT, xT):
        # vaug[:, jt, h*65 : h*65+64] = (x_v @ Wv.T)[jt-tile, h*64:(h+1)*64] + bv
        for jt in range(TT):
            for oc in range(2):          # 384 columns (6 heads) per matmul
                if oc == 0:
                    ps = ps_pool.tile([P, 1024], FP, tag="s", name=f"pv{jt}_{oc}")
                else:
                    ps = pt_pool.tile([P, 512], FP, tag="t", name=f"pv{jt}_{oc}")
                psv = ps[:, :384]
                for je in range(ET):
                    nc.tensor.matmul(
                        psv,
                        xT[:, je, jt * P:(jt + 1) * P],
                        wT[:, je, oc * 384:(oc + 1) * 384],
                        start=(je == 0), stop=(je == ET - 1),
                    )
                out3 = vaug[:, jt].rearrange("p (h x) -> p h x", x=VW)
                out3 = out3[:, oc * 6:(oc + 1) * 6, 0:64]
                nc.vector.tensor_add(
                    out3,
                    psv.rearrange("p (h x) -> p h x", x=64),
                    bv_b[:, oc * 384:(oc + 1) * 384].rearrange(
                        "p (h x) -> p h x", x=64),
                )

    wT = wT_pool.tile([P, ET, E], CDT, tag="w")
    xT = xT_pool.tile([P, ET, T], CDT, tag="x")
    build_T(xT, io["query"], T)
    build_T(wT, io["Wq"], E)
    proj_headmajor(qT, wT, xT, bqt)

    wT = wT_pool.tile([P, ET, E], CDT, tag="w")
    xT = xT_pool.tile([P, ET, T], CDT, tag="x")
    build_T(xT, io["key"], T)
    build_T(wT, io["Wk"], E)
    proj_headmajor(kT, wT, xT, bkt)

    wT = wT_pool.tile([P, ET, E], CDT, tag="w")
    xT = xT_pool.tile([P, ET, T], CDT, tag="x")
    build_T(xT, io["value"], T)
    build_T(wT, io["Wv"], E)
    proj_v(wT, xT)

    woT = wT_pool.tile([P, ET, E], CDT, tag="w")
    build_T(woT, io["Wo"], E)

    # ---- attention: per (head-pair, tq-half) block, pipelined epilogue ---
    cT = cT_pool.tile([P, ET, T], CDT)  # normalized (attn @ v).T, head-major
    vaug3 = vaug.rearrange("p jt (h x) -> p jt h x", x=VW)

    blocks = [(pr, th) for pr in range(NPAIR) for th in range(2)]

    def epilogue_start(blk):
        """Normalizer chain + U.T -> cT for a finished block (off PE path)."""
        pr, th, u_ps, ets = blk["pr"], blk["th"], blk["u"], blk["ets"]
        tq0 = th * 512
        blk["rcol"] = []
        for h2 in range(2):
            rl = rl_pool.tile([P, 1028], FP, tag="rl",
                              name=f"rl{pr}_{th}_{h2}")
            srow = rl[0:1, 0:512]
            rcol = rl[:, 512:516]
            rrow = rl[0:1, 516:1028]
            rr = rr_pool.tile([64, 512], FP, tag="rr",
                              name=f"rr{pr}_{th}_{h2}")
            # rowsum row -> SBUF (plain ACT copy: no table-set switch)
            nc.scalar.copy(srow, u_ps[h2][64:65, :])
            # reciprocal row (~18-bit custom-DVE op), broadcast to 64 rows
            nc.vector.reciprocal_approx_fast(out=rrow, in_=srow)
            nc.gpsimd.partition_broadcast(rr[:, :], rrow)
            # normalized head output -> cT (frees the u psum slot)
            nc.vector.tensor_mul(
                cT[h2 * 64:(h2 + 1) * 64, pr, tq0:tq0 + 512],
                u_ps[h2][0:64, :],
                rr[:, :],
            )
            # column form: tiny PE transposes of the rowsum row, exact recip
            lc = pt_pool.tile([P, 4], FP, tag="t", name=f"lc{pr}_{th}_{h2}")
            for jq in range(4):
                nc.tensor.matmul(
                    lc[:, jq:jq + 1],
                    srow[0:1, jq * P:(jq + 1) * P],
                    one_f32[:],
                    start=True, stop=True,
                )
            nc.vector.reciprocal(rcol, lc[:])
            blk["rcol"].append(rcol)

    def epilogue_group(blk, g):
        """Transpose+evict+store one (h2, jq) attn stripe of a finished block."""
        pr, th, ets = blk["pr"], blk["th"], blk["ets"]
        h2, jq = divmod(g, 4)
        h = 2 * pr + h2
        # REGULAR matmul-by-identity (not transpose-mode): transpose-mode
        # doesn't register as PE-busy for the HAM clock monitor, which
        # throttles the PE to half clock. fp32 PSUM out is the price.
        t_ps = ps_pool.tile([P, T], FP, tag="s", name=f"tp{pr}_{th}_{g}")
        for jk in range(TT):
            nc.tensor.matmul(
                t_ps[:, jk * P:(jk + 1) * P],
                ets[jk][:, h2 * 512 + jq * P: h2 * 512 + (jq + 1) * P],
                ident[:],
                start=True, stop=True,
            )
        # the SWDGE DMA widens bf16 -> fp32 on the way to HBM
        ast = ast_pool.tile([P, T], CDT, tag="ast", name=f"as{pr}_{th}_{g}")
        rc = blk["rcol"][h2][:, jq:jq + 1]
        if h2 == 0:
            nc.scalar.mul(ast[:], t_ps[:], rc)
        else:
            nc.vector.tensor_scalar_mul(ast[:], t_ps[:], rc)
        nc.gpsimd.dma_start(
            io["attn"][h, (th * 4 + jq) * P:(th * 4 + jq + 1) * P, :],
            ast[:],
        )

    pending = None
    for pr, th in blocks:
        tq0 = th * 512
        u_ps = [
            pu_pool.tile([VW, 512], FP, tag="u", name=f"u{pr}_{th}_{i}")
            for i in range(2)
        ]
        if pending is not None:
            epilogue_start(pending)
        ets = []
        for jk in range(TT):
            s_ps = ps_pool.tile([P, 1024], FP, tag="s", name=f"s{pr}_{th}_{jk}")
            for h2 in range(2):
                nc.tensor.matmul(
                    s_ps[:, h2 * 512:(h2 + 1) * 512],
                    kT[h2 * 64:(h2 + 1) * 64, pr, jk * P:(jk + 1) * P],
                    qT[h2 * 64:(h2 + 1) * 64, pr, tq0:tq0 + 512],
                    start=True, stop=True,
                )
            et = et_pool.tile([P, 1024], CDT, tag="et", name=f"e{pr}_{th}_{jk}")
            nc.scalar.activation(et[:], s_ps[:], AF.Exp, scale=SCALE)
            ets.append(et)
            for h2 in range(2):
                nc.tensor.matmul(
                    u_ps[h2][:],
                    vaug3[:, jk, 2 * pr + h2, :],
                    et[:, h2 * 512:(h2 + 1) * 512],
                    start=(jk == 0), stop=(jk == TT - 1),
                )
            if pending is not None:
                epilogue_group(pending, jk)
        pending = {"pr": pr, "th": th, "u": u_ps, "ets": ets}

    # flush the last block's epilogue
    epilogue_start(pending)
    for g in range(8):
        epilogue_group(pending, g)

    # ---- output projection ----------------------------------------------
    wT = woT
    for jt in range(TT):
        yst = y_pool.tile([P, E], FP, tag="y", name=f"y{jt}")
        for oc in range(2):
            if oc == 0:
                ps = ps_pool.tile([P, 1024], FP, tag="s", name=f"py{jt}_{oc}")
            else:
                ps = pt_pool.tile([P, 512], FP, tag="t", name=f"py{jt}_{oc}")
            psv = ps[:, :384]
            for je in range(ET):
                nc.tensor.matmul(
                    psv,
                    cT[:, je, jt * P:(jt + 1) * P],
                    wT[:, je, oc * 384:(oc + 1) * 384],
                    start=(je == 0), stop=(je == ET - 1),
                )
            nc.vector.tensor_add(
                yst[:, oc * 384:(oc + 1) * 384], psv,
                bo_b[:, oc * 384:(oc + 1) * 384],
            )
        nc.sync.dma_start(io["out"][jt * P:(jt + 1) * P, :], yst[:])


def build_nc():
    nc = bacc.Bacc("TRN2", target_bir_lowering=False, debug=False)
    io = {}
    for name in ("query", "key", "value"):
        io[name] = nc.dram_tensor(name, [T, E], FP, kind="ExternalInput").ap()
    for name in ("Wq", "Wk", "Wv", "Wo"):
        io[name] = nc.dram_tensor(name, [E, E], FP, kind="ExternalInput").ap()
    for name in ("bq", "bk", "bv", "bo"):
        io[name] = nc.dram_tensor(name, [E], FP, kind="ExternalInput").ap()
    io["out"] = nc.dram_tensor("out", [T, E], FP, kind="ExternalOutput").ap()
    io["attn"] = nc.dram_tensor("attn", [H, T, T], FP, kind="ExternalOutput").ap()

    with tile.TileContext(nc) as tc:
        with ExitStack() as ctx:
            _emit(ctx, tc, io)
    nc.compile()
    return nc


_NC_CACHE = None


def _get_nc():
    global _NC_CACHE
    if _NC_CACHE is None:
        _NC_CACHE = build_nc()
    return _NC_CACHE


def kernel(query, key, value, Wq, bq, Wk, bk, Wv, bv, Wo, bo, **bass_run_kwargs):
    from concourse.bass_utils import run_bass_kernel_spmd

    nc = _get_nc()
    f32 = lambda a: np.ascontiguousarray(np.asarray(a), dtype=np.float32)
    shared = {
        "Wq": f32(Wq), "bq": f32(bq), "Wk": f32(Wk), "bk": f32(bk),
        "Wv": f32(Wv), "bv": f32(bv), "Wo": f32(Wo), "bo": f32(bo),
    }
    q, k, v = f32(query), f32(key), f32(value)
    in_maps = [
        {"query": q[c], "key": k[c], "value": v[c], **shared} for c in range(NC)
    ]
    res = run_bass_kernel_spmd(nc, in_maps, list(range(NC)), **bass_run_kwargs)
    out = np.stack([res.results[c]["out"] for c in range(NC)])
    attn = np.stack([res.results[c]["attn"] for c in range(NC)])
    if bass_run_kwargs:
        return (out, attn), res
    return (out, attn)


# revision 16
# speedup vs baseline: 1.0755x; 1.0755x over previous
"""Multi-head attention (B=8, N=1024, E=768, H=12) on 8 trn2 NeuronCores.

Sharding: pure data-parallel over batch — core c computes batch element c
entirely (QKV projections, per-head attention, output projection), so no
collectives are needed. Returns (out, attn) exactly like the reference.

Per-core layout strategy (bf16 matmul inputs, fp32 PSUM accumulation,
fp32 softmax normalization):
  - PE contracts over the partition axis, so x (tokens-major) and W
    ([out,in]) are transposed on-chip (PE transpose-mode), giving
    x.T / W.T in bf16.
  - q.T, k.T are head-major ([o, t]); one 128-partition o-tile holds a
    PAIR of heads, so the K=64 score matmuls for the two heads run
    concurrently in disjoint PE row-groups.
  - Scores are computed TRANSPOSED (S.T = k @ q.T per head): softmax's
    reduction axis (tk) lands on partitions, where a matmul against an
    appended ones-column of V computes the softmax denominators for free:
    lhsT = [v_h | 1] gives U.T rows 0..63 = E'v and row 64 = rowsums.
  - exp runs on ScalarE straight out of PSUM (scale=1/sqrt(dh) folded in),
    emitting bf16 E.T tiles.
  - Normalizers come from an ACT ln/exp chain (no slow 1-lane DVE
    reciprocal): ln(rowsum) row -> tiny PE transposes -> exp(-ln) gives the
    per-tq reciprocal in both row and column form.
  - Attention probabilities: E.T tiles are transposed back through the PE
    (transpose-mode, bf16 PSUM) and evicted PSUM->SBUF on VectorE at 2x
    mode with the per-partition reciprocal scale.
  - Each block's epilogue (transposes/evictions/DMA) is software-pipelined
    into the NEXT block's score/exp/AV loop so the PE stream stays dense.
"""

import numpy as np
from contextlib import ExitStack

import concourse.bass as bass
import concourse.mybir as mybir
import concourse.tile as tile
from concourse import bacc
from concourse.masks import make_identity

P = 128
T = 1024          # tokens
E = 768           # embed dim
H = 12            # heads
DH = 64           # head dim
SCALE = DH ** -0.5
ET = E // P       # 6 e-tiles
TT = T // P       # 8 t-tiles
NPAIR = H // 2    # 6 head pairs
NC = 8            # cores
VW = 65           # v columns per head in vaug (64 + ones column)

FP = mybir.dt.float32
CDT = mybir.dt.bfloat16     # matmul-input dtype (PSUM accumulation is fp32)
AF = mybir.ActivationFunctionType


def _emit(ctx: ExitStack, tc: tile.TileContext, io: dict):
    nc = tc.nc

    const = ctx.enter_context(tc.tile_pool(name="const", bufs=1))
    nat_pool = ctx.enter_context(tc.tile_pool(name="nat", bufs=4))
    xT_pool = ctx.enter_context(tc.tile_pool(name="xT", bufs=2))
    wT_pool = ctx.enter_context(tc.tile_pool(name="wT", bufs=2))
    qT_pool = ctx.enter_context(tc.tile_pool(name="qT", bufs=1))
    kT_pool = ctx.enter_context(tc.tile_pool(name="kT", bufs=1))
    cT_pool = ctx.enter_context(tc.tile_pool(name="cT", bufs=1))
    vaug_pool = ctx.enter_context(tc.tile_pool(name="vaug", bufs=1))
    et_pool = ctx.enter_context(tc.tile_pool(name="et", bufs=18))
    ast_pool = ctx.enter_context(tc.tile_pool(name="ast", bufs=4))
    y_pool = ctx.enter_context(tc.tile_pool(name="y", bufs=2))
    rl_pool = ctx.enter_context(tc.tile_pool(name="rl", bufs=4))
    rr_pool = ctx.enter_context(tc.tile_pool(name="rr", bufs=4))
    ps_pool = ctx.enter_context(tc.tile_pool(name="ps", bufs=2, space="PSUM"))
    pt_pool = ctx.enter_context(tc.tile_pool(name="pt", bufs=2, space="PSUM"))
    pu_pool = ctx.enter_context(tc.tile_pool(name="pu", bufs=2, space="PSUM"))

    # ---- constants: identity, biases ------------------------------------
    ident = const.tile([P, P], CDT)
    make_identity(nc, ident[:])
    one_f32 = const.tile([1, 1], FP)
    nc.any.memset(one_f32[:], 1.0)

    bqt = const.tile([P, ET], FP)   # bq as per-partition columns per o-tile
    nc.sync.dma_start(bqt[:], io["bq"].rearrange("(o p) -> p o", p=P))
    bkt = const.tile([P, ET], FP)
    nc.sync.dma_start(bkt[:], io["bk"].rearrange("(o p) -> p o", p=P))

    bv_row = const.tile([1, E], FP)
    nc.sync.dma_start(bv_row[:], io["bv"].rearrange("(a e) -> a e", a=1))
    bv_b = const.tile([P, E], FP)
    nc.gpsimd.partition_broadcast(bv_b[:], bv_row[:])
    bo_row = const.tile([1, E], FP)
    nc.sync.dma_start(bo_row[:], io["bo"].rearrange("(a e) -> a e", a=1))
    bo_b = const.tile([P, E], FP)
    nc.gpsimd.partition_broadcast(bo_b[:], bo_row[:])

    # ---- helper: build transposed copy in SBUF --------------------------
    def build_T(dst, src_dram, R):
        """dst sbuf [P, CT, R] <- transpose of src_dram [R, C=CT*128].

        dst[p, c, r] = src[r, c*128+p]."""
        CT = dst.shape[1]
        for r in range(R // P):
            nat = nat_pool.tile([P, CT * P], CDT, tag="nat")
            # SWDGE DMA casts fp32 -> bf16 in flight
            nc.gpsimd.dma_start(nat[:], src_dram[r * P:(r + 1) * P, :])
            ps = pu_pool.tile([P, CT * P], CDT, tag="u", name=f"bt{r}")
            for c in range(CT):
                nc.tensor.transpose(
                    ps[:, c * P:(c + 1) * P],
                    nat[:, c * P:(c + 1) * P],
                    ident[:],
                )
            nc.vector.tensor_copy(
                dst[:, :, r * P:(r + 1) * P],
                ps[:].rearrange("p (c x) -> p c x", x=P),
            )

    # ---- projections ----------------------------------------------------
    # q.T / k.T: [P, ET, T] head-pair-major: dst[p, jo, t] = proj.T[jo*128+p, t]
    qT = qT_pool.tile([P, ET, T], CDT)
    kT = kT_pool.tile([P, ET, T], CDT)
    # vaug: [P, TT, H*65]: per t-tile, per head: 64 v columns + ones column
    vaug = vaug_pool.tile([P, TT, H * VW], CDT)
    nc.any.memset(vaug[:], 1.0)

    def proj_headmajor(dst, wT, xT, bias_cols):
        # dst[:, jo, t] = (x @ W.T).T rows [jo*128, jo*128+128) + bias
        for jo in range(ET):
            for t2 in range(2):
                if (jo * 2 + t2) % 2 == 0:
                    ps = ps_pool.tile([P, 1024], FP, tag="s", name=f"pp{jo}_{t2}")
                else:
                    ps = pt_pool.tile([P, 512], FP, tag="t", name=f"pp{jo}_{t2}")
                psv = ps[:, :512]
                for je in range(ET):
                    nc.tensor.matmul(
                        psv,
                        wT[:, je, jo * P:(jo + 1) * P],
                        xT[:, je, t2 * 512:(t2 + 1) * 512],
                        start=(je == 0), stop=(je == ET - 1),
                    )
                nc.vector.tensor_scalar_add(
                    dst[:, jo, t2 * 512:(t2 + 1) * 512], psv,
                    bias_cols[:, jo:jo + 1],
                )

    def proj_v(wT, xT):
        # vaug[:, jt, h*65 : h*65+64] = (x_v @ Wv.T)[jt-tile, h*64:(h+1)*64] + bv
        for jt in range(TT):
            for oc in range(2):          # 384 columns (6 heads) per matmul
                if oc == 0:
                    ps = ps_pool.tile([P, 1024], FP, tag="s", name=f"pv{jt}_{oc}")
                else:
                    ps = pt_pool.tile([P, 512], FP, tag="t", name=f"pv{jt}_{oc}")
                psv = ps[:, :384]
                for je in range(ET):
                    nc.tensor.matmul(
                        psv,
                        xT[:, je, jt * P:(jt + 1) * P],
                        wT[:, je, oc * 384:(oc + 1) * 384],
                        start=(je == 0), stop=(je == ET - 1),
                    )
                out3 = vaug[:, jt].rearrange("p (h x) -> p h x", x=VW)
                out3 = out3[:, oc * 6:(oc + 1) * 6, 0:64]
                nc.vector.tensor_add(
                    out3,
                    psv.rearrange("p (h x) -> p h x", x=64),
                    bv_b[:, oc * 384:(oc + 1) * 384].rearrange(
                        "p (h x) -> p h x", x=64),
                )

    wT = wT_pool.tile([P, ET, E], CDT, tag="w")
    xT = xT_pool.tile([P, ET, T], CDT, tag="x")
    build_T(xT, io["query"], T)
    build_T(wT, io["Wq"], E)
    proj_headmajor(qT, wT, xT, bqt)

    wT = wT_pool.tile([P, ET, E], CDT, tag="w")
    xT = xT_pool.tile([P, ET, T], CDT, tag="x")
    build_T(xT, io["key"], T)
    build_T(wT, io["Wk"], E)
    proj_headmajor(kT, wT, xT, bkt)

    wT = wT_pool.tile([P, ET, E], CDT, tag="w")
    xT = xT_pool.tile([P, ET, T], CDT, tag="x")
    build_T(xT, io["value"], T)
    build_T(wT, io["Wv"], E)
    proj_v(wT, xT)

    woT = wT_pool.tile([P, ET, E], CDT, tag="w")
    build_T(woT, io["Wo"], E)

    # ---- attention: per (head-pair, tq-half) block, pipelined epilogue ---
    cT = cT_pool.tile([P, ET, T], CDT)  # normalized (attn @ v).T, head-major
    vaug3 = vaug.rearrange("p jt (h x) -> p jt h x", x=VW)

    blocks = [(pr, th) for pr in range(NPAIR) for th in range(2)]

    def epilogue_start(blk):
        """Normalizer chain + U.T -> cT for a finished block (off PE path)."""
        pr, th, u_ps, ets = blk["pr"], blk["th"], blk["u"], blk["ets"]
        tq0 = th * 512
        blk["rcol"] = []
        for h2 in range(2):
            rl = rl_pool.tile([P, 1028], FP, tag="rl",
                              name=f"rl{pr}_{th}_{h2}")
            srow = rl[0:1, 0:512]
            rcol = rl[:, 512:516]
            rrow = rl[0:1, 516:1028]
            rr = rr_pool.tile([64, 512], FP, tag="rr",
                              name=f"rr{pr}_{th}_{h2}")
            # rowsum row -> SBUF (plain ACT copy: no table-set switch)
            nc.scalar.copy(srow, u_ps[h2][64:65, :])
            # reciprocal row (~18-bit custom-DVE op), broadcast to 64 rows
            nc.vector.reciprocal_approx_fast(out=rrow, in_=srow)
            nc.gpsimd.partition_broadcast(rr[:, :], rrow)
            # normalized head output -> cT (frees the u psum slot)
            nc.vector.tensor_mul(
                cT[h2 * 64:(h2 + 1) * 64, pr, tq0:tq0 + 512],
                u_ps[h2][0:64, :],
                rr[:, :],
            )
            # column form: tiny PE transposes of the rowsum row, exact recip
            lc = pt_pool.tile([P, 4], FP, tag="t", name=f"lc{pr}_{th}_{h2}")
            for jq in range(4):
                nc.tensor.matmul(
                    lc[:, jq:jq + 1],
                    srow[0:1, jq * P:(jq + 1) * P],
                    one_f32[:],
                    start=True, stop=True,
                )
            nc.vector.reciprocal(rcol, lc[:])
            blk["rcol"].append(rcol)

    def epilogue_group(blk, g):
        """Transpose+evict+store one (h2, jq) attn stripe of a finished block."""
        pr, th, ets = blk["pr"], blk["th"], blk["ets"]
        h2, jq = divmod(g, 4)
        h = 2 * pr + h2
        t_ps = pt_pool.tile([P, T], CDT, tag="t", name=f"tp{pr}_{th}_{g}")
        for jk in range(TT):
            nc.tensor.transpose(
                t_ps[:, jk * P:(jk + 1) * P],
                ets[jk][:, h2 * 512 + jq * P: h2 * 512 + (jq + 1) * P],
                ident[:],
            )
        # bf16 in + bf16 out hits the DVE 2x perf mode; the SWDGE DMA
        # widens bf16 -> fp32 on the way to HBM.
        ast = ast_pool.tile([P, T], CDT, tag="ast", name=f"as{pr}_{th}_{g}")
        nc.vector.tensor_scalar_mul(ast[:], t_ps[:], blk["rcol"][h2][:, jq:jq + 1])
        nc.gpsimd.dma_start(
            io["attn"][h, (th * 4 + jq) * P:(th * 4 + jq + 1) * P, :],
            ast[:],
        )

    pending = None
    for pr, th in blocks:
        tq0 = th * 512
        u_ps = [
            pu_pool.tile([VW, 512], FP, tag="u", name=f"u{pr}_{th}_{i}")
            for i in range(2)
        ]
        if pending is not None:
            epilogue_start(pending)
        ets = []
        for jk in range(TT):
            s_ps = ps_pool.tile([P, 1024], FP, tag="s", name=f"s{pr}_{th}_{jk}")
            for h2 in range(2):
                nc.tensor.matmul(
                    s_ps[:, h2 * 512:(h2 + 1) * 512],
                    kT[h2 * 64:(h2 + 1) * 64, pr, jk * P:(jk + 1) * P],
                    qT[h2 * 64:(h2 + 1) * 64, pr, tq0:tq0 + 512],
                    start=True, stop=True,
                )
            et = et_pool.tile([P, 1024], CDT, tag="et", name=f"e{pr}_{th}_{jk}")
            nc.scalar.activation(et[:], s_ps[:], AF.Exp, scale=SCALE)
            ets.append(et)
            for h2 in range(2):
                nc.tensor.matmul(
                    u_ps[h2][:],
                    vaug3[:, jk, 2 * pr + h2, :],
                    et[:, h2 * 512:(h2 + 1) * 512],
                    start=(jk == 0), stop=(jk == TT - 1),
                )
            if pending is not None:
                epilogue_group(pending, jk)
        pending = {"pr": pr, "th": th, "u": u_ps, "ets": ets}

    # flush the last block's epilogue
    epilogue_start(pending)
    for g in range(8):
        epilogue_group(pending, g)

    # ---- output projection ----------------------------------------------
    wT = woT
    for jt in range(TT):
        yst = y_pool.tile([P, E], FP, tag="y", name=f"y{jt}")
        for oc in range(2):
            if oc == 0:
                ps = ps_pool.tile([P, 1024], FP, tag="s", name=f"py{jt}_{oc}")
            else:
                ps = pt_pool.tile([P, 512], FP, tag="t", name=f"py{jt}_{oc}")
            psv = ps[:, :384]
            for je in range(ET):
                nc.tensor.matmul(
                    psv,
                    cT[:, je, jt * P:(jt + 1) * P],
                    wT[:, je, oc * 384:(oc + 1) * 384],
                    start=(je == 0), stop=(je == ET - 1),
                )
            nc.vector.tensor_add(
                yst[:, oc * 384:(oc + 1) * 384], psv,
                bo_b[:, oc * 384:(oc + 1) * 384],
            )
        nc.sync.dma_start(io["out"][jt * P:(jt + 1) * P, :], yst[:])


def build_nc():
    nc = bacc.Bacc("TRN2", target_bir_lowering=False, debug=False)
    io = {}
    for name in ("query", "key", "value"):
        io[name] = nc.dram_tensor(name, [T, E], FP, kind="ExternalInput").ap()
    for name in ("Wq", "Wk", "Wv", "Wo"):
        io[name] = nc.dram_tensor(name, [E, E], FP, kind="ExternalInput").ap()
    for name in ("bq", "bk", "bv", "bo"):
        io[name] = nc.dram_tensor(name, [E], FP, kind="ExternalInput").ap()
    io["out"] = nc.dram_tensor("out", [T, E], FP, kind="ExternalOutput").ap()
    io["attn"] = nc.dram_tensor("attn", [H, T, T], FP, kind="ExternalOutput").ap()

    with tile.TileContext(nc) as tc:
        with ExitStack() as ctx:
            _emit(ctx, tc, io)
    nc.compile()
    return nc


_NC_CACHE = None


def _get_nc():
    global _NC_CACHE
    if _NC_CACHE is None:
        _NC_CACHE = build_nc()
    return _NC_CACHE


def kernel(query, key, value, Wq, bq, Wk, bk, Wv, bv, Wo, bo, **bass_run_kwargs):
    from concourse.bass_utils import run_bass_kernel_spmd

    nc = _get_nc()
    f32 = lambda a: np.ascontiguousarray(np.asarray(a), dtype=np.float32)
    shared = {
        "Wq": f32(Wq), "bq": f32(bq), "Wk": f32(Wk), "bk": f32(bk),
        "Wv": f32(Wv), "bv": f32(bv), "Wo": f32(Wo), "bo": f32(bo),
    }
    q, k, v = f32(query), f32(key), f32(value)
    in_maps = [
        {"query": q[c], "key": k[c], "value": v[c], **shared} for c in range(NC)
    ]
    res = run_bass_kernel_spmd(nc, in_maps, list(range(NC)), **bass_run_kwargs)
    out = np.stack([res.results[c]["out"] for c in range(NC)])
    attn = np.stack([res.results[c]["attn"] for c in range(NC)])
    if bass_run_kwargs:
        return (out, attn), res
    return (out, attn)


# revision 18
# speedup vs baseline: 1.1071x; 1.0294x over previous
"""Multi-head attention (B=8, N=1024, E=768, H=12) on 8 trn2 NeuronCores.

Sharding: pure data-parallel over batch — core c computes batch element c
entirely (QKV projections, per-head attention, output projection), so no
collectives are needed. Returns (out, attn) exactly like the reference.

Per-core layout strategy (bf16 matmul inputs, fp32 PSUM accumulation,
fp32 softmax normalization):
  - PE contracts over the partition axis, so x (tokens-major) and W
    ([out,in]) are transposed on-chip (PE transpose-mode), giving
    x.T / W.T in bf16.
  - q.T, k.T are head-major ([o, t]); one 128-partition o-tile holds a
    PAIR of heads, so the K=64 score matmuls for the two heads run
    concurrently in disjoint PE row-groups.
  - Scores are computed TRANSPOSED (S.T = k @ q.T per head): softmax's
    reduction axis (tk) lands on partitions, where a matmul against an
    appended ones-column of V computes the softmax denominators for free:
    lhsT = [v_h | 1] gives U.T rows 0..63 = E'v and row 64 = rowsums.
  - exp runs on ScalarE straight out of PSUM (scale=1/sqrt(dh) folded in),
    emitting bf16 E.T tiles.
  - Normalizers come from an ACT ln/exp chain (no slow 1-lane DVE
    reciprocal): ln(rowsum) row -> tiny PE transposes -> exp(-ln) gives the
    per-tq reciprocal in both row and column form.
  - Attention probabilities: E.T tiles are transposed back through the PE
    (transpose-mode, bf16 PSUM) and evicted PSUM->SBUF on VectorE at 2x
    mode with the per-partition reciprocal scale.
  - Each block's epilogue (transposes/evictions/DMA) is software-pipelined
    into the NEXT block's score/exp/AV loop so the PE stream stays dense.
"""

import numpy as np
from contextlib import ExitStack

import concourse.bass as bass
import concourse.mybir as mybir
import concourse.tile as tile
from concourse import bacc
from concourse.masks import make_identity

P = 128
T = 1024          # tokens
E = 768           # embed dim
H = 12            # heads
DH = 64           # head dim
SCALE = DH ** -0.5
ET = E // P       # 6 e-tiles
TT = T // P       # 8 t-tiles
NPAIR = H // 2    # 6 head pairs
NC = 8            # cores
VW = 65           # v columns per head in vaug (64 + ones column)

FP = mybir.dt.float32
CDT = mybir.dt.bfloat16     # matmul-input dtype (PSUM accumulation is fp32)
AF = mybir.ActivationFunctionType


def _emit(ctx: ExitStack, tc: tile.TileContext, io: dict):
    nc = tc.nc

    const = ctx.enter_context(tc.tile_pool(name="const", bufs=1))
    nat_pool = ctx.enter_context(tc.tile_pool(name="nat", bufs=4))
    xT_pool = ctx.enter_context(tc.tile_pool(name="xT", bufs=2))
    wT_pool = ctx.enter_context(tc.tile_pool(name="wT", bufs=2))
    qT_pool = ctx.enter_context(tc.tile_pool(name="qT", bufs=1))
    kT_pool = ctx.enter_context(tc.tile_pool(name="kT", bufs=1))
    cT_pool = ctx.enter_context(tc.tile_pool(name="cT", bufs=1))
    vaug_pool = ctx.enter_context(tc.tile_pool(name="vaug", bufs=1))
    et_pool = ctx.enter_context(tc.tile_pool(name="et", bufs=18))
    ast_pool = ctx.enter_context(tc.tile_pool(name="ast", bufs=4))
    y_pool = ctx.enter_context(tc.tile_pool(name="y", bufs=2))
    rl_pool = ctx.enter_context(tc.tile_pool(name="rl", bufs=4))
    rr_pool = ctx.enter_context(tc.tile_pool(name="rr", bufs=4))
    ps_pool = ctx.enter_context(tc.tile_pool(name="ps", bufs=2, space="PSUM"))
    pt_pool = ctx.enter_context(tc.tile_pool(name="pt", bufs=2, space="PSUM"))
    pu_pool = ctx.enter_context(tc.tile_pool(name="pu", bufs=2, space="PSUM"))

    # ---- constants: identity, biases ------------------------------------
    ident = const.tile([P, P], CDT)
    make_identity(nc, ident[:])
    one_f32 = const.tile([1, 1], FP)
    nc.any.memset(one_f32[:], 1.0)

    bqt = const.tile([P, ET], FP)   # bq as per-partition columns per o-tile
    nc.sync.dma_start(bqt[:], io["bq"].rearrange("(o p) -> p o", p=P))
    bkt = const.tile([P, ET], FP)
    nc.sync.dma_start(bkt[:], io["bk"].rearrange("(o p) -> p o", p=P))

    bv_row = const.tile([1, E], FP)
    nc.sync.dma_start(bv_row[:], io["bv"].rearrange("(a e) -> a e", a=1))
    bv_b = const.tile([P, E], FP)
    nc.gpsimd.partition_broadcast(bv_b[:], bv_row[:])
    bo_row = const.tile([1, E], FP)
    nc.sync.dma_start(bo_row[:], io["bo"].rearrange("(a e) -> a e", a=1))
    bo_b = const.tile([P, E], FP)
    nc.gpsimd.partition_broadcast(bo_b[:], bo_row[:])

    # ---- helper: build transposed copy in SBUF --------------------------
    def build_T(dst, src_dram, R):
        """dst sbuf [P, CT, R] <- transpose of src_dram [R, C=CT*128].

        dst[p, c, r] = src[r, c*128+p]."""
        CT = dst.shape[1]
        for r in range(R // P):
            nat = nat_pool.tile([P, CT * P], CDT, tag="nat")
            # SWDGE DMA casts fp32 -> bf16 in flight
            nc.gpsimd.dma_start(nat[:], src_dram[r * P:(r + 1) * P, :])
            ps = pu_pool.tile([P, CT * P], CDT, tag="u", name=f"bt{r}")
            for c in range(CT):
                nc.tensor.transpose(
                    ps[:, c * P:(c + 1) * P],
                    nat[:, c * P:(c + 1) * P],
                    ident[:],
                )
            if r % 2 == 0:
                nc.vector.tensor_copy(
                    dst[:, :, r * P:(r + 1) * P],
                    ps[:].rearrange("p (c x) -> p c x", x=P),
                )
            else:
                nc.scalar.copy(
                    dst[:, :, r * P:(r + 1) * P],
                    ps[:].rearrange("p (c x) -> p c x", x=P),
                )

    # ---- projections ----------------------------------------------------
    # q.T / k.T: [P, ET, T] head-pair-major: dst[p, jo, t] = proj.T[jo*128+p, t]
    qT = qT_pool.tile([P, ET, T], CDT)
    kT = kT_pool.tile([P, ET, T], CDT)
    # vaug: [P, TT, H*65]: per t-tile, per head: 64 v columns + ones column
    vaug = vaug_pool.tile([P, TT, H * VW], CDT)
    nc.any.memset(vaug[:], 1.0)

    def proj_headmajor(dst, wT, xT, bias_cols):
        # dst[:, jo, t] = (x @ W.T).T rows [jo*128, jo*128+128) + bias
        for jo in range(ET):
            for t2 in range(2):
                if (jo * 2 + t2) % 2 == 0:
                    ps = ps_pool.tile([P, 1024], FP, tag="s", name=f"pp{jo}_{t2}")
                else:
                    ps = pt_pool.tile([P, 512], FP, tag="t", name=f"pp{jo}_{t2}")
                psv = ps[:, :512]
                for je in range(ET):
                    nc.tensor.matmul(
                        psv,
                        wT[:, je, jo * P:(jo + 1) * P],
                        xT[:, je, t2 * 512:(t2 + 1) * 512],
                        start=(je == 0), stop=(je == ET - 1),
                    )
                nc.vector.tensor_scalar_add(
                    dst[:, jo, t2 * 512:(t2 + 1) * 512], psv,
                    bias_cols[:, jo:jo + 1],
                )

    def proj_v(wT, xT):
        # vaug[:, jt, h*65 : h*65+64] = (x_v @ Wv.T)[jt-tile, h*64:(h+1)*64] + bv
        for jt in range(TT):
            for oc in range(2):          # 384 columns (6 heads) per matmul
                if oc == 0:
                    ps = ps_pool.tile([P, 1024], FP, tag="s", name=f"pv{jt}_{oc}")
                else:
                    ps = pt_pool.tile([P, 512], FP, tag="t", name=f"pv{jt}_{oc}")
                psv = ps[:, :384]
                for je in range(ET):
                    nc.tensor.matmul(
                        psv,
                        xT[:, je, jt * P:(jt + 1) * P],
                        wT[:, je, oc * 384:(oc + 1) * 384],
                        start=(je == 0), stop=(je == ET - 1),
                    )
                out3 = vaug[:, jt].rearrange("p (h x) -> p h x", x=VW)
                out3 = out3[:, oc * 6:(oc + 1) * 6, 0:64]
                nc.vector.tensor_add(
                    out3,
                    psv.rearrange("p (h x) -> p h x", x=64),
                    bv_b[:, oc * 384:(oc + 1) * 384].rearrange(
                        "p (h x) -> p h x", x=64),
                )

    wT = wT_pool.tile([P, ET, E], CDT, tag="w")
    xT = xT_pool.tile([P, ET, T], CDT, tag="x")
    build_T(xT, io["query"], T)
    build_T(wT, io["Wq"], E)
    proj_headmajor(qT, wT, xT, bqt)

    wT = wT_pool.tile([P, ET, E], CDT, tag="w")
    xT = xT_pool.tile([P, ET, T], CDT, tag="x")
    build_T(xT, io["key"], T)
    build_T(wT, io["Wk"], E)
    proj_headmajor(kT, wT, xT, bkt)

    wT = wT_pool.tile([P, ET, E], CDT, tag="w")
    xT = xT_pool.tile([P, ET, T], CDT, tag="x")
    build_T(xT, io["value"], T)
    build_T(wT, io["Wv"], E)
    proj_v(wT, xT)

    woT = wT_pool.tile([P, ET, E], CDT, tag="w")
    build_T(woT, io["Wo"], E)

    # ---- attention: per (head-pair, tq-half) block, pipelined epilogue ---
    cT = cT_pool.tile([P, ET, T], CDT)  # normalized (attn @ v).T, head-major
    vaug3 = vaug.rearrange("p jt (h x) -> p jt h x", x=VW)

    blocks = [(pr, th) for pr in range(NPAIR) for th in range(2)]

    def epilogue_start(blk):
        """Normalizer chain + U.T -> cT for a finished block (off PE path)."""
        pr, th, u_ps, ets = blk["pr"], blk["th"], blk["u"], blk["ets"]
        tq0 = th * 512
        blk["rcol"] = []
        for h2 in range(2):
            rl = rl_pool.tile([P, 1028], FP, tag="rl",
                              name=f"rl{pr}_{th}_{h2}")
            srow = rl[0:1, 0:512]
            rcol = rl[:, 512:516]
            rrow = rl[0:1, 516:1028]
            rr = rr_pool.tile([64, 512], FP, tag="rr",
                              name=f"rr{pr}_{th}_{h2}")
            # rowsum row -> SBUF (plain ACT copy: no table-set switch)
            nc.scalar.copy(srow, u_ps[h2][64:65, :])
            # reciprocal row (~18-bit custom-DVE op), broadcast to 64 rows
            nc.vector.reciprocal_approx_fast(out=rrow, in_=srow)
            nc.gpsimd.partition_broadcast(rr[:, :], rrow)
            # normalized head output -> cT (frees the u psum slot)
            nc.vector.tensor_mul(
                cT[h2 * 64:(h2 + 1) * 64, pr, tq0:tq0 + 512],
                u_ps[h2][0:64, :],
                rr[:, :],
            )
            # column form: tiny PE transposes of the rowsum row, exact recip
            lc = pt_pool.tile([P, 4], FP, tag="t", name=f"lc{pr}_{th}_{h2}")
            for jq in range(4):
                nc.tensor.matmul(
                    lc[:, jq:jq + 1],
                    srow[0:1, jq * P:(jq + 1) * P],
                    one_f32[:],
                    start=True, stop=True,
                )
            nc.vector.reciprocal(rcol, lc[:])
            blk["rcol"].append(rcol)

    def group_transposes(blk, g, lo, hi):
        """PE transposes [lo, hi) of one (h2, jq) attn stripe.

        Transpose-mode ops don't count as PE-busy for the HAM clock
        monitor, so these are interleaved between regular matmuls by the
        caller to keep the PE clock at full rate."""
        pr, th, ets = blk["pr"], blk["th"], blk["ets"]
        h2, jq = divmod(g, 4)
        if lo == 0:
            blk["tps"] = pt_pool.tile([P, T], CDT, tag="t",
                                      name=f"tp{pr}_{th}_{g}")
        t_ps = blk["tps"]
        for jk in range(lo, hi):
            nc.tensor.transpose(
                t_ps[:, jk * P:(jk + 1) * P],
                ets[jk][:, h2 * 512 + jq * P: h2 * 512 + (jq + 1) * P],
                ident[:],
            )

    def group_evict(blk, g):
        """Evict+store one finished attn stripe (bf16 2x DVE; SWDGE DMA
        widens bf16 -> fp32 on the way to HBM)."""
        pr, th = blk["pr"], blk["th"]
        h2, jq = divmod(g, 4)
        h = 2 * pr + h2
        t_ps = blk["tps"]
        ast = ast_pool.tile([P, T], CDT, tag="ast", name=f"as{pr}_{th}_{g}")
        nc.vector.tensor_scalar_mul(ast[:], t_ps[:], blk["rcol"][h2][:, jq:jq + 1])
        nc.gpsimd.dma_start(
            io["attn"][h, (th * 4 + jq) * P:(th * 4 + jq + 1) * P, :],
            ast[:],
        )

    pending = None
    for pr, th in blocks:
        tq0 = th * 512
        u_ps = [
            pu_pool.tile([VW, 512], FP, tag="u", name=f"u{pr}_{th}_{i}")
            for i in range(2)
        ]
        if pending is not None:
            epilogue_start(pending)
        ets = []
        for jk in range(TT):
            if pending is not None:
                group_transposes(pending, jk, 0, 4)
            s_ps = ps_pool.tile([P, 1024], FP, tag="s", name=f"s{pr}_{th}_{jk}")
            for h2 in range(2):
                nc.tensor.matmul(
                    s_ps[:, h2 * 512:(h2 + 1) * 512],
                    kT[h2 * 64:(h2 + 1) * 64, pr, jk * P:(jk + 1) * P],
                    qT[h2 * 64:(h2 + 1) * 64, pr, tq0:tq0 + 512],
                    start=True, stop=True,
                )
            et = et_pool.tile([P, 1024], CDT, tag="et", name=f"e{pr}_{th}_{jk}")
            nc.scalar.activation(et[:], s_ps[:], AF.Exp, scale=SCALE)
            ets.append(et)
            for h2 in range(2):
                nc.tensor.matmul(
                    u_ps[h2][:],
                    vaug3[:, jk, 2 * pr + h2, :],
                    et[:, h2 * 512:(h2 + 1) * 512],
                    start=(jk == 0), stop=(jk == TT - 1),
                )
            if pending is not None:
                group_transposes(pending, jk, 4, TT)
                group_evict(pending, jk)
        pending = {"pr": pr, "th": th, "u": u_ps, "ets": ets}

    # ---- output projection, interleaved with the last block's epilogue ---
    epilogue_start(pending)
    wT = woT
    for jt in range(TT):
        group_transposes(pending, jt, 0, 4)
        yst = y_pool.tile([P, E], FP, tag="y", name=f"y{jt}")
        for oc in range(2):
            if oc == 0:
                ps = ps_pool.tile([P, 1024], FP, tag="s", name=f"py{jt}_{oc}")
            else:
                ps = pt_pool.tile([P, 512], FP, tag="t", name=f"py{jt}_{oc}")
            psv = ps[:, :384]
            for je in range(ET):
                nc.tensor.matmul(
                    psv,
                    cT[:, je, jt * P:(jt + 1) * P],
                    wT[:, je, oc * 384:(oc + 1) * 384],
                    start=(je == 0), stop=(je == ET - 1),
                )
            nc.vector.tensor_add(
                yst[:, oc * 384:(oc + 1) * 384], psv,
                bo_b[:, oc * 384:(oc + 1) * 384],
            )
            if oc == 0:
                group_transposes(pending, jt, 4, TT)
                group_evict(pending, jt)
        nc.sync.dma_start(io["out"][jt * P:(jt + 1) * P, :], yst[:])


def build_nc():
    nc = bacc.Bacc("TRN2", target_bir_lowering=False, debug=False)
    io = {}
    for name in ("query", "key", "value"):
        io[name] = nc.dram_tensor(name, [T, E], FP, kind="ExternalInput").ap()
    for name in ("Wq", "Wk", "Wv", "Wo"):
        io[name] = nc.dram_tensor(name, [E, E], FP, kind="ExternalInput").ap()
    for name in ("bq", "bk", "bv", "bo"):
        io[name] = nc.dram_tensor(name, [E], FP, kind="ExternalInput").ap()
    io["out"] = nc.dram_tensor("out", [T, E], FP, kind="ExternalOutput").ap()
    io["attn"] = nc.dram_tensor("attn", [H, T, T], FP, kind="ExternalOutput").ap()

    with tile.TileContext(nc) as tc:
        with ExitStack() as ctx:
            _emit(ctx, tc, io)
    nc.compile()
    return nc


_NC_CACHE = None


def _get_nc():
    global _NC_CACHE
    if _NC_CACHE is None:
        _NC_CACHE = build_nc()
    return _NC_CACHE


def kernel(query, key, value, Wq, bq, Wk, bk, Wv, bv, Wo, bo, **bass_run_kwargs):
    from concourse.bass_utils import run_bass_kernel_spmd

    nc = _get_nc()
    f32 = lambda a: np.ascontiguousarray(np.asarray(a), dtype=np.float32)
    shared = {
        "Wq": f32(Wq), "bq": f32(bq), "Wk": f32(Wk), "bk": f32(bk),
        "Wv": f32(Wv), "bv": f32(bv), "Wo": f32(Wo), "bo": f32(bo),
    }
    q, k, v = f32(query), f32(key), f32(value)
    in_maps = [
        {"query": q[c], "key": k[c], "value": v[c], **shared} for c in range(NC)
    ]
    res = run_bass_kernel_spmd(nc, in_maps, list(range(NC)), **bass_run_kwargs)
    out = np.stack([res.results[c]["out"] for c in range(NC)])
    attn = np.stack([res.results[c]["attn"] for c in range(NC)])
    if bass_run_kwargs:
        return (out, attn), res
    return (out, attn)
